# revision 11
# baseline (speedup 1.0000x reference)
"""Trainium2 Bass kernel for NonGridProjectLayer (camera projection + bilinear sampling).

Contract: kernel(**inputs) takes FULL inputs (as produced by setup_inputs) and
returns the FULL output tuple (feats [N,P,V,C] f32, bounding [N,P] bool,
sample_points [N,P,5] f32).

Design
------
Work unit: a (n, v, half-of-P) "segment" of 2048 sample points. There are
N*V*2 = 40 segments; each of the 8 cores gets exactly 5 (balanced).

Per core, on device:
  * Points live as [128 partitions, 80 cols] f32 planes (point t -> partition
    t%128, column t//128). Per-segment camera constants are pre-broadcast by
    the host into [128, 80] "const planes" so every step of the projection /
    distortion / affine / grid math is a plain elementwise DVE op.
  * The bilinear sample indices (4 corner rows of a [H*W, C] feature-map
    table) and weights (incl. validity + in-image bound masks) are computed
    on device in f32, converted to int32.
  * An indirect DMA (SWDGE gather) fetches corner rows (512B each) from the
    DRAM table; a fused tensor_scalar / scalar_tensor_tensor chain applies
    the 4 bilinear weights; results DMA out as [128, 80, 128].

The host only reshapes/transposes inputs into per-core tables+planes and
reassembles the output (pure data marshaling; all math on device).
"""

import os
import sys

for _p in ("/opt/trn_rl_repo", "/root/.axon_site/_ro/trn_rl_repo"):
    if os.path.isdir(_p) and _p not in sys.path:
        sys.path.insert(0, _p)

import numpy as np

import concourse.bass as bass
import concourse.bacc as bacc
import concourse.tile as tile
from concourse import mybir
from concourse.bass import IndirectOffsetOnAxis

F32 = mybir.dt.float32
I32 = mybir.dt.int32
A = mybir.AluOpType

# Problem shape (hardcoded per spec)
N, V, C, H, W, P = 4, 5, 128, 128, 128, 4096
HW = H * W
NCORES = 8
NSEG = 5                    # segments per core
SEG_PTS = P // 2            # 2048 points per segment
PTS = NSEG * SEG_PTS        # 10240 points per core
COLS = PTS // 128           # 80
SEGCOLS = SEG_PTS // 128    # 16
NMAPS = 3                   # distinct feature maps per core (by construction)
ROWS = NMAPS * HW           # rows in the per-core gather table
NP = 38                     # number of const planes
CPG = 2                     # chunk-columns (128 points each) per gather DMA
SHIFT = 64.0                # grid coords are shifted +64 so floor() input > 0

# ---------------------------------------------------------------------------
# Segment -> core assignment
# ---------------------------------------------------------------------------

def _core_segments():
    """Returns per-core list of (pair, half); pair = n*V + v."""
    units = [(pair, half) for pair in range(N * V) for half in (0, 1)]
    return [units[c * NSEG:(c + 1) * NSEG] for c in range(NCORES)]


def _core_maps(segs):
    """Ordered unique pairs for one core -> map slot assignment."""
    maps = []
    for pair, _ in segs:
        if pair not in maps:
            maps.append(pair)
    assert len(maps) <= NMAPS
    return maps


# ---------------------------------------------------------------------------
# Device program (SPMD: identical for all cores)
# ---------------------------------------------------------------------------

_PROGRAM = None


def build_program(debug=False, reps=1):
    nc = bacc.Bacc("TRN2")
    table = nc.declare_dram_parameter("table", [ROWS, C], F32, isOutput=False)
    ptsd = nc.declare_dram_parameter("pts", [128, 3 * COLS], F32, isOutput=False)
    cond = nc.declare_dram_parameter("consts", [128, NP * COLS], F32, isOutput=False)
    featsd = nc.declare_dram_parameter("feats", [128, COLS, C], F32, isOutput=True)
    boundd = nc.declare_dram_parameter("bound", [128, COLS], F32, isOutput=True)
    if debug:
        idxd = nc.declare_dram_parameter("idx4_out", [128, COLS, 2], I32,
                                         isOutput=True)
        w4d = nc.declare_dram_parameter("w4_out", [128, COLS, 4], F32,
                                        isOutput=True)
        gbd = nc.declare_dram_parameter("gb_out", [128, CPG * 4 * C], F32,
                                        isOutput=True)

    with tile.TileContext(nc) as tc:
        with (
            tc.tile_pool(name="work", bufs=1) as wp,
            tc.tile_pool(name="gat", bufs=4) as gp,
            tc.tile_pool(name="outp", bufs=4) as op_,
        ):
          def _body():
            ct = wp.tile([128, NP * COLS], F32, name="consts")
            nc.sync.dma_start(out=ct[:], in_=cond[:])
            pt = wp.tile([128, 3 * COLS], F32, name="ptsb")
            nc.sync.dma_start(out=pt[:], in_=ptsd[:])

            def CP(i):  # const plane i as [128, COLS] AP
                return ct[:, i * COLS:(i + 1) * COLS]

            xs = pt[:, 0:COLS]
            ys = pt[:, COLS:2 * COLS]
            zs = pt[:, 2 * COLS:3 * COLS]

            ctr = [0]

            def newt(dtype=F32):
                ctr[0] += 1
                return wp.tile([128, COLS], dtype, name=f"t{ctr[0]}")

            def TT(op, a, b, out=None):
                o = out if out is not None else newt()
                nc.vector.tensor_tensor(out=o, in0=a, in1=b, op=op)
                return o

            def TS(a, s1, op0, s2=None, op1=None, out=None):
                o = out if out is not None else newt()
                if s2 is None:
                    nc.vector.tensor_scalar(o, a, float(s1), None, op0)
                else:
                    nc.vector.tensor_scalar(o, a, float(s1), float(s2), op0, op1)
                return o

            def STT(a, s, b, op0, op1, out=None):
                o = out if out is not None else newt()
                nc.vector.scalar_tensor_tensor(
                    out=o, in0=a, scalar=float(s), in1=b, op0=op0, op1=op1)
                return o

            # --- camera: Xc = R@X + (-R@T) -------------------------------
            Xc = []
            for i in range(3):
                s = TT(A.mult, xs, CP(3 * i + 0))
                u = TT(A.mult, ys, CP(3 * i + 1))
                s = TT(A.add, s, u)
                u2 = TT(A.mult, zs, CP(3 * i + 2))
                s = TT(A.add, s, u2)
                s = TT(A.add, s, CP(9 + i))
                Xc.append(s)

            rz = newt()
            nc.vector.reciprocal(out=rz, in_=Xc[2])
            # one Newton step: rz = rz * (2 - Xc2*rz)
            e = TT(A.mult, Xc[2], rz)
            e = TS(e, -1.0, A.mult, 2.0, A.add, out=e)
            rz = TT(A.mult, rz, e, out=rz)
            xn = TT(A.mult, Xc[0], rz)
            yn = TT(A.mult, Xc[1], rz)

            # --- distortion ---------------------------------------------
            xn2 = TT(A.mult, xn, xn)
            yn2 = TT(A.mult, yn, yn)
            r2 = TT(A.add, xn2, yn2)
            xyp = TT(A.mult, xn, yn)
            rad = TT(A.mult, r2, CP(18))            # k3*r2
            rad = TT(A.add, rad, CP(17), out=rad)   # +k2
            rad = TT(A.mult, rad, r2, out=rad)
            rad = TT(A.add, rad, CP(16), out=rad)   # +k1
            rad = TT(A.mult, rad, r2, out=rad)
            rad = TS(rad, 1.0, A.add, out=rad)      # +1

            xd = TT(A.mult, xn, rad)
            u = TT(A.mult, xyp, CP(21))             # 2*p1*xn*yn
            xd = TT(A.add, xd, u, out=xd)
            t = STT(xn2, 2.0, r2, A.mult, A.add)    # r2 + 2*xn^2
            t = TT(A.mult, t, CP(20), out=t)        # *p2
            xd = TT(A.add, xd, t, out=xd)

            yd = TT(A.mult, yn, rad)
            u2 = TT(A.mult, xyp, CP(22))            # 2*p2*xn*yn
            yd = TT(A.add, yd, u2, out=yd)
            t2 = STT(yn2, 2.0, r2, A.mult, A.add)   # r2 + 2*yn^2
            t2 = TT(A.mult, t2, CP(19), out=t2)     # *p1
            yd = TT(A.add, yd, t2, out=yd)

            # --- pixel coords + bound ------------------------------------
            px = TT(A.mult, xd, CP(12))
            px = TT(A.add, px, CP(14), out=px)
            py = TT(A.mult, yd, CP(13))
            py = TT(A.add, py, CP(15), out=py)

            b1 = TS(px, 0.0, A.is_ge)
            b2 = TS(py, 0.0, A.is_ge)
            b3 = TT(A.is_lt, px, CP(23))
            b4 = TT(A.is_lt, py, CP(24))
            bound = TT(A.mult, b1, b2)
            b34 = TT(A.mult, b3, b4)
            bound = TT(A.mult, bound, b34, out=bound)
            nc.sync.dma_start(out=boundd[:], in_=bound)

            # --- clip pixel, affine to feature map, normalize to grid ----
            pxc = TS(px, -1.0, A.max)
            pxc = TT(A.min, pxc, CP(25), out=pxc)
            pyc = TS(py, -1.0, A.max)
            pyc = TT(A.min, pyc, CP(25), out=pyc)

            fmx = TT(A.mult, pxc, CP(26))
            u3 = TT(A.mult, pyc, CP(27))
            fmx = TT(A.add, fmx, u3, out=fmx)
            fmx = TT(A.add, fmx, CP(28), out=fmx)
            fmy = TT(A.mult, pxc, CP(29))
            u4 = TT(A.mult, pyc, CP(30))
            fmy = TT(A.add, fmy, u4, out=fmy)
            fmy = TT(A.add, fmy, CP(31), out=fmy)

            # gx = (clip(fmx*sx - 1, -1.1, 1.1) + 1) * (W-1)/2 + SHIFT
            gx = TT(A.mult, fmx, CP(32))
            gx = TS(gx, -1.0, A.add, -1.1, A.max, out=gx)
            gx = TS(gx, 1.1, A.min, 1.0, A.add, out=gx)
            gx = TS(gx, (W - 1) / 2.0, A.mult, SHIFT, A.add, out=gx)
            gy = TT(A.mult, fmy, CP(33))
            gy = TS(gy, -1.0, A.add, -1.1, A.max, out=gy)
            gy = TS(gy, 1.1, A.min, 1.0, A.add, out=gy)
            gy = TS(gy, (H - 1) / 2.0, A.mult, SHIFT, A.add, out=gy)

            # --- floor (rounding-mode agnostic) --------------------------
            def floor_(g):
                gi = newt(I32)
                nc.vector.tensor_copy(out=gi, in_=g)
                gf = newt()
                nc.vector.tensor_copy(out=gf, in_=gi)
                d = TT(A.is_gt, gf, g)
                return STT(d, -1.0, gf, A.mult, A.add)

            x0 = floor_(gx)          # shifted: true x0 = x0 - 64
            y0 = floor_(gy)
            wx1 = TT(A.subtract, gx, x0)
            wx0 = TS(wx1, -1.0, A.mult, 1.0, A.add)
            wy1 = TT(A.subtract, gy, y0)
            wy0 = TS(wy1, -1.0, A.mult, 1.0, A.add)

            # --- clip indices + validity ---------------------------------
            # The gather fetches PAIRS of consecutive table rows: base
            # bx = clip(x0, 63, 190) -> rows (y, bx), (y, bx+1). The sx
            # switch remaps weights when x0=191 (pair base shifts left).
            lo, hi = SHIFT, SHIFT + W - 1
            xc0 = TS(x0, lo, A.max, hi, A.min)
            vx0 = TT(A.is_equal, x0, xc0)
            xv1 = TS(x0, lo - 1, A.max, hi - 1, A.min)
            vx1 = TT(A.is_equal, x0, xv1)                # x1 in-range?
            bx = TS(x0, lo, A.max, hi - 1, A.min)        # pair base
            yc0 = TS(y0, lo, A.max, hi, A.min)
            vy0 = TT(A.is_equal, y0, yc0)
            yc1 = TS(y0, lo - 1, A.max, hi - 1, A.min)   # y1 row - 1
            vy1 = TT(A.is_equal, y0, yc1)
            # pair slots: A=row(y,bx), B=row(y,bx+1). Normally A<-x0, B<-x1;
            # at x0=lo-1 the pair shifts right (A<-x1), at x0=hi it shifts
            # left (B<-x0). sL/sH select those cases (vx masks the rest).
            sL = TS(x0, lo - 1, A.is_le)
            sH = TS(x0, hi, A.is_ge)
            sM = TT(A.add, sL, sH)
            sM = TS(sM, -1.0, A.mult, 1.0, A.add, out=sM)

            # --- final weights (validity, bound, pair-remap folded) ------
            wx0v = TT(A.mult, wx0, vx0)
            wx1v = TT(A.mult, wx1, vx1)
            wxA = TT(A.mult, wx0v, sM)
            t0 = TT(A.mult, wx1v, sL)
            wxA = TT(A.add, wxA, t0, out=wxA)
            wxB = TT(A.mult, wx1v, sM)
            u0 = TT(A.mult, wx0v, sH)
            wxB = TT(A.add, wxB, u0, out=wxB)
            wy0v = TT(A.mult, wy0, vy0)
            wy0v = TT(A.mult, wy0v, bound, out=wy0v)
            wy1v = TT(A.mult, wy1, vy1)
            wy1v = TT(A.mult, wy1v, bound, out=wy1v)

            w4 = wp.tile([128, COLS, 4], F32, name="w4")
            TT(A.mult, wxA, wy0v, out=w4[:, :, 0])
            TT(A.mult, wxB, wy0v, out=w4[:, :, 1])
            TT(A.mult, wxA, wy1v, out=w4[:, :, 2])
            TT(A.mult, wxB, wy1v, out=w4[:, :, 3])

            # --- pair-base row indices -----------------------------------
            # true row = (yc-64)*W + (bx-64) + slot*HW; plane 34 holds
            # slot*HW - 64*W - 64, plane 35 the same + W (y1 gather).
            idx2f = wp.tile([128, COLS, 2], F32, name="idx2f")
            r0 = STT(yc0, float(W), bx, A.mult, A.add)
            TT(A.add, r0, CP(34), out=idx2f[:, :, 0])
            r1 = STT(yc1, float(W), bx, A.mult, A.add)
            TT(A.add, r1, CP(36), out=idx2f[:, :, 1])

            idx2 = wp.tile([128, COLS, 2], I32, name="idx2")
            nc.vector.tensor_copy(out=idx2[:], in_=idx2f[:])
            if debug:
                nc.sync.dma_start(out=idxd[:], in_=idx2[:])
                nc.sync.dma_start(out=w4d[:], in_=w4[:])

            # --- gather (one offset per partition, 2-row units) ----------
            for g in range(COLS // CPG):
                c0 = g * CPG
                gb = gp.tile([128, CPG * 4 * C], F32, name="gbuf")
                for j in range(CPG):
                    for yi in (0, 1):
                        nc.gpsimd.indirect_dma_start(
                            out=gb[:, (j * 2 + yi) * 2 * C:
                                    (j * 2 + yi + 1) * 2 * C],
                            out_offset=None,
                            in_=table[:],
                            in_offset=IndirectOffsetOnAxis(
                                ap=idx2[:, c0 + j, yi:yi + 1], axis=0),
                        )
                if debug and g == 0:
                    nc.sync.dma_start(out=gbd[:], in_=gb[:])
                ob = op_.tile([128, CPG, C], F32, name="obuf")
                for j in range(CPG):
                    cc = c0 + j

                    def corner(ci):
                        return gb[:, (j * 4 + ci) * C:(j * 4 + ci + 1) * C]

                    nc.vector.tensor_scalar(
                        ob[:, j, :], corner(0), w4[:, cc, 0:1], None, A.mult)
                    for ci in (1, 2, 3):
                        nc.vector.scalar_tensor_tensor(
                            out=ob[:, j, :], in0=corner(ci),
                            scalar=w4[:, cc, ci:ci + 1], in1=ob[:, j, :],
                            op0=A.mult, op1=A.add)
                nc.sync.dma_start(out=featsd[:, c0:c0 + CPG, :], in_=ob[:])

          if reps == 1:
              _body()
          else:
              with tc.For_i(0, reps, 1):
                  _body()

    return nc


def get_program():
    global _PROGRAM
    if _PROGRAM is None:
        nc = build_program()
        if not nc.is_finalized():
            nc.finalize()
        _PROGRAM = nc
    return _PROGRAM


# ---------------------------------------------------------------------------
# Host marshaling
# ---------------------------------------------------------------------------

def marshal(feature_maps, sample_points, R, T, f, c, k, p, trans, wh, fm_size):
    fm = np.ascontiguousarray(np.asarray(feature_maps, np.float32))
    sp = np.asarray(sample_points, np.float32)
    R = np.asarray(R, np.float32)
    T = np.asarray(T, np.float32)
    f = np.asarray(f, np.float32)
    c = np.asarray(c, np.float32)
    k = np.asarray(k, np.float32)
    pp = np.asarray(p, np.float32)
    trans = np.asarray(trans, np.float32)
    wh = np.asarray(wh, np.float32)
    fm_size = np.asarray(fm_size, np.float32)

    core_segs = _core_segments()
    in_maps = []
    for cid in range(NCORES):
        segs = core_segs[cid]
        maps = _core_maps(segs)
        table = np.empty((ROWS, C), np.float32)
        for slot, pair in enumerate(maps):
            n, v = divmod(pair, V)
            table[slot * HW:(slot + 1) * HW] = (
                fm[n, v].transpose(1, 2, 0).reshape(HW, C))
        pts = np.empty((128, 3 * COLS), np.float32)
        consts = np.empty((128, NP * COLS), np.float32)
        for s, (pair, half) in enumerate(segs):
            n, v = divmod(pair, V)
            p0 = half * SEG_PTS
            cs = slice(s * SEGCOLS, (s + 1) * SEGCOLS)
            for j in range(3):
                blk = sp[n, p0:p0 + SEG_PTS, j].reshape(SEGCOLS, 128)
                pts[:, j * COLS:(j + 1) * COLS][:, cs] = blk.T
            slot = maps.index(pair)
            base = float(slot * HW - SHIFT * W - SHIFT)
            Rnv = R[n, v].astype(np.float64)
            Tnv = T[n, v].astype(np.float64)
            sc = np.zeros(NP, np.float64)
            sc[0:9] = Rnv.reshape(9)
            sc[9:12] = -(Rnv @ Tnv)
            sc[12:14] = f[n, v]
            sc[14:16] = c[n, v]
            sc[16:19] = k[n, v]
            sc[19] = pp[n, v, 0]
            sc[20] = pp[n, v, 1]
            sc[21] = 2.0 * pp[n, v, 0]
            sc[22] = 2.0 * pp[n, v, 1]
            sc[23] = wh[n, v, 0]
            sc[24] = wh[n, v, 1]
            sc[25] = max(wh[n, v, 0], wh[n, v, 1])
            sc[26:32] = trans[n, v].reshape(6)
            sc[32] = 2.0 / (float(fm_size[0]) - 1.0)
            sc[33] = 2.0 / (float(fm_size[1]) - 1.0)
            sc[34] = base
            sc[35] = base + 1
            sc[36] = base + W
            sc[37] = base + W + 1
            # (34 = y0 pair base, 36 = y1 pair base; 35/37 unused now)
            scf = sc.astype(np.float32)
            for kk in range(NP):
                consts[:, kk * COLS:(kk + 1) * COLS][:, cs] = scf[kk]
        in_maps.append({"table": table, "pts": pts, "consts": consts})
    return in_maps


def assemble(results, sample_points):
    sp = np.asarray(sample_points, np.float32)
    core_segs = _core_segments()
    feats = np.empty((N, P, V, C), np.float32)
    bound_nv = np.empty((N, V, P), np.float32)
    for cid in range(NCORES):
        fo = np.asarray(results[cid]["feats"])   # [128, COLS, C]
        bo = np.asarray(results[cid]["bound"])   # [128, COLS]
        for s, (pair, half) in enumerate(core_segs[cid]):
            n, v = divmod(pair, V)
            p0 = half * SEG_PTS
            cs = slice(s * SEGCOLS, (s + 1) * SEGCOLS)
            feats[n, p0:p0 + SEG_PTS, v, :] = (
                fo[:, cs, :].transpose(1, 0, 2).reshape(SEG_PTS, C))
            bound_nv[n, v, p0:p0 + SEG_PTS] = bo[:, cs].T.reshape(SEG_PTS)
    bounding = (bound_nv.sum(axis=1) > 0) & ~np.isnan(feats).any(axis=(2, 3))
    return feats, bounding, sp


# ---------------------------------------------------------------------------
# Entry point
# ---------------------------------------------------------------------------

def kernel(**inputs):
    from concourse.bass_utils import run_bass_kernel_spmd

    nc = get_program()
    in_maps = marshal(**inputs)
    res = run_bass_kernel_spmd(nc, in_maps, list(range(NCORES)))
    return assemble(res.results, inputs["sample_points"])


if __name__ == "__main__":
    # smoke: build the program only
    get_program()
    print("program built ok")
